# revision 1
# baseline (speedup 1.0000x reference)
"""AdaptiveMoDBlock Trainium2 kernel — gate-aware mixed fp8/bf16 variant.

Host: routing/top-k as in the reference; selected tokens are then SORTED by
gate ascending and dealt round-robin to the 8 cores (each core sees the same
gate-quantile profile, so one SPMD graph fits all).  The output error a token
contributes scales with gate^2, so low-gate tokens run the inner FFN in e4m3
DoubleRow (2 MACs/PE/cycle) and high-gate tokens in bf16; a boundary chunk
mixes precision across the contraction (first mA/mB k-doubles fp8, rest bf16)
to hit the error budget exactly.

Per token chunk j (width Nj <= 512) the compile-time config is (mA_j of 8,
mB_j of 32) = number of 256-row k-doubles computed in fp8 for the two FFN
matmuls; remaining contraction rows run in bf16.  Device layouts per chunk:
  x8_j  f8  [128, mA_j, 2, Nj]    x8[p,t,o,n]  = sel[n, (2t+o)*128+p]
  x16_j bf16[128, 16-2mA_j, Nj]   rows 2mA_j..15
  h8_j  f8  [128, mB_j, 2, Nj]    dff rows < 2mB_j*128 (phase-A output)
  h16_j bf16[128, 64-2mB_j, Nj]   dff rows >= 2mB_j*128
Weights (shared): w1t8/w2t8 fp8 (x64 pre-scale), w1t16/w2t16 bf16 tails.
"""

import math
import sys

import numpy as np

sys.path.insert(0, "/opt/trn_rl_repo")

import ml_dtypes  # noqa: E402

BF16 = ml_dtypes.bfloat16
F8 = ml_dtypes.float8_e4m3   # IEEE e4m3 (max 240) == TRN FP8_EXP4

B, S, D = 4, 4096, 2048
DC = D // 4
DFF = 4 * D
MIN_CAP, MAX_CAP = 0.25, 1.0

NCORES = 8
TPAD = 256
NMAX = 256
WSCALE = 64.0
TARGET_MASS = 0.54   # fraction of gate^2 mass allowed in fp8
SWI = True           # software-interleaved DoubleRow weight layout

_cache = {}


def _token_chunks(n_tok):
    chunks = []
    off = 0
    while off < n_tok:
        w = min(NMAX, n_tok - off)
        chunks.append((off, w))
        off += w
    return chunks


def _choose_cfg(n_tok, gate_sorted):
    """Per-chunk (mA, mB) from the global sorted gate^2 mass curve."""
    chunks = _token_chunks(n_tok)
    g2 = gate_sorted.astype(np.float64) ** 2
    tot = g2.sum()
    if tot <= 0:
        return tuple((8, 32) for _ in chunks)
    cfg = []
    rem = TARGET_MASS
    for off, N in chunks:
        m = g2[NCORES * off: NCORES * (off + N)].sum() / tot
        if m <= rem:
            phi = 1.0
            rem -= m
        elif rem > 0:
            phi = rem / m
            rem = 0.0
        else:
            phi = 0.0
        # bias the boundary chunk toward phase-B fp8: same error mass, but
        # larger mB shrinks the bf16 h footprint (SBUF is tight)
        if phi > 0.0:
            mB = min(32, int(phi * 32) + 3)
            mA = max(0, min(8, int((phi - mB / 64.0) * 16.0 + 1e-9)))
        else:
            mA = mB = 0
        cfg.append((mA, mB))
    return tuple(cfg)


def _layout(n_tok, gate_sorted):
    """Merge adjacent same-precision 256-chunks into <=512-wide chunks
    (SwInterleave hides LDWEIGHTS, so wider moving spans just amortize the
    per-matmul issue overhead)."""
    chunks = _token_chunks(n_tok)
    cfg = _choose_cfg(n_tok, gate_sorted)
    merged = []
    for (off, N), (mA, mB) in zip(chunks, cfg):
        if (merged and merged[-1][2] == mA and merged[-1][3] == mB
                and merged[-1][1] + N <= 512
                and merged[-1][0] + merged[-1][1] == off):
            merged[-1] = (merged[-1][0], merged[-1][1] + N, mA, mB)
        else:
            merged.append((off, N, mA, mB))
    return tuple(merged)


def _build(n_tok, layout):
    """Build + compile the per-core FFN graph."""
    from concourse import bacc, mybir, tile

    F32 = mybir.dt.float32
    FP8 = mybir.dt.float8e4
    BF = mybir.dt.bfloat16
    AF = mybir.ActivationFunctionType
    DR = (mybir.MatmulPerfMode.DoubleRowSwInterleave if SWI else
          mybir.MatmulPerfMode.DoubleRow)

    chunks = [(o, n) for (o, n, a, b) in layout]
    nch = len(chunks)
    mAs = [a for (o, n, a, b) in layout]
    mBs = [b for (o, n, a, b) in layout]
    nb1 = 16 - 2 * min(mAs)          # bf16 w1 rows (in 128-chunks)
    nb2 = 64 - 2 * min(mBs)          # bf16 w2 rows

    nc = bacc.Bacc("TRN2", target_bir_lowering=False, debug=False,
                   num_devices=NCORES)
    x8t, x16t = [], []
    for j, (off, N) in enumerate(chunks):
        mA = mAs[j]
        x8t.append(nc.declare_dram_parameter(f"x8_{j}", [128, mA, 2, N], FP8,
                                             isOutput=False) if mA else None)
        x16t.append(nc.declare_dram_parameter(
            f"x16_{j}", [128, 16 - 2 * mA, N], BF,
            isOutput=False) if mA < 8 else None)
    w1t8 = nc.declare_dram_parameter("w1t8", [64, 128, 8, 2, 128], FP8,
                                     isOutput=False)
    w1t16 = (nc.declare_dram_parameter("w1t16", [64, 128, nb1, 128], BF,
                                       isOutput=False) if nb1 else None)
    w2t8 = nc.declare_dram_parameter("w2t8", [16, 128, 32, 2, 128], FP8,
                                     isOutput=False)
    w2t16 = (nc.declare_dram_parameter("w2t16", [16, 128, nb2, 128], BF,
                                       isOutput=False) if nb2 else None)
    b1t = nc.declare_dram_parameter("b1t", [128, 64], F32, isOutput=False)
    b2t = nc.declare_dram_parameter("b2t", [128, 16], F32, isOutput=False)
    wtok = nc.declare_dram_parameter("wtok", [128, n_tok], BF, isOutput=False)
    out = nc.declare_dram_parameter("out", [16, 128, n_tok], BF,
                                    isOutput=True)

    with tile.TileContext(nc) as tc:
        with (
            tc.tile_pool(name="const", bufs=1) as cpool,
            tc.tile_pool(name="w1p8", bufs=3) as w1p8,
            tc.tile_pool(name="w1p16", bufs=3) as w1p16,
            tc.tile_pool(name="w2p8", bufs=3) as w2p8,
            tc.tile_pool(name="w2p16", bufs=3) as w2p16,
            tc.tile_pool(name="op", bufs=2) as op,
            tc.tile_pool(name="ps", bufs=8, space="PSUM") as psp,
        ):
            x8_sb, x16_sb, h8_sb, h16_sb = [], [], [], []
            for j, (off, N) in enumerate(chunks):
                mA, mB = mAs[j], mBs[j]
                x8_sb.append(cpool.tile([128, mA, 2, N], FP8, name=f"x8s{j}",
                                        tag=f"x8s{j}") if mA else None)
                x16_sb.append(cpool.tile([128, 16 - 2 * mA, N], BF,
                                         name=f"x16s{j}", tag=f"x16s{j}")
                              if mA < 8 else None)
                h8_sb.append(cpool.tile([128, mB, 2, N], FP8, name=f"h8s{j}",
                                        tag=f"h8s{j}") if mB else None)
                h16_sb.append(cpool.tile([128, 64 - 2 * mB, N], BF,
                                         name=f"h16s{j}", tag=f"h16s{j}")
                              if mB < 32 else None)

            # ramp DMA: first two w1 slab pairs, then x pieces alternating
            # between the scalar HWDGE ring and gpsimd SWDGE queues, ordered
            # by PE consumption (chunk-major, k ascending).
            w18_pre, w116_pre = {}, {}
            t8 = w1p8.tile([128, 8, 2, 128], FP8, name="w18s", tag="w18s")
            nc.sync.dma_start(t8[:], w1t8[0])
            w18_pre[0] = t8
            # x pieces round-robin over all three rings in PE consumption
            # order; the first chunk's pieces go out before the w116
            # preloads so the c=0 chain is never queued behind slabs
            pieces = []
            for j in range(nch):
                mA = mAs[j]
                for t in range(mA):
                    pieces.append((x8_sb[j][:, t], x8t[j][:, t]))
                if mA < 8:
                    for t in range(16 - 2 * mA):
                        pieces.append((x16_sb[j][:, t], x16t[j][:, t]))
            rings = [nc.sync, nc.scalar, nc.gpsimd]
            nfirst = min(len(pieces), mAs[0] if mAs[0] else 4)
            for ri, (dst, srcp) in enumerate(pieces[:nfirst]):
                rings[ri % 3].dma_start(dst, srcp)
            t8 = w1p8.tile([128, 8, 2, 128], FP8, name="w18s", tag="w18s")
            nc.sync.dma_start(t8[:], w1t8[1])
            w18_pre[1] = t8
            if w1t16 is not None:
                for c, ring in ((0, nc.scalar), (1, nc.gpsimd)):
                    t16 = w1p16.tile([128, nb1, 128], BF, name="w116s",
                                     tag="w116s")
                    ring.dma_start(t16[:], w1t16[c])
                    w116_pre[c] = t16
            for ri, (dst, srcp) in enumerate(pieces[nfirst:]):
                rings[ri % 3].dma_start(dst, srcp)
            b1_sb = cpool.tile([128, 64], F32, tag="b1")
            nc.gpsimd.dma_start(b1_sb[:], b1t[:])
            wtok_sb = cpool.tile([128, n_tok], BF, tag="wtok")
            nc.gpsimd.dma_start(wtok_sb[:], wtok[:])
            b2_sb = cpool.tile([128, 16], F32, tag="b2")
            nc.gpsimd.dma_start(b2_sb[:], b2t[:])

            # phase A
            for c in range(64):
                if c in w18_pre:
                    w18_sb = w18_pre.pop(c)
                    w116_sb = w116_pre.pop(c, None)
                else:
                    w18_sb = w1p8.tile([128, 8, 2, 128], FP8, name="w18s",
                                       tag="w18s")
                    nc.sync.dma_start(w18_sb[:], w1t8[c])
                    w116_sb = None
                    if w1t16 is not None:
                        w116_sb = w1p16.tile([128, nb1, 128], BF,
                                             name="w116s", tag="w116s")
                        ring = nc.scalar if c % 2 == 0 else nc.gpsimd
                        ring.dma_start(w116_sb[:], w1t16[c])
                for j, (off, N) in enumerate(chunks):
                    mA = mAs[j]
                    ps = psp.tile([128, 512], F32, name="ps", tag="ps")
                    nmm = mA + (16 - 2 * mA)
                    i = 0
                    for t in range(mA):
                        nc.tensor.matmul(
                            ps[:, :N], w18_sb[:, t], x8_sb[j][:, t],
                            start=(i == 0), stop=(i == nmm - 1), perf_mode=DR)
                        i += 1
                    for tc_ in range(2 * mA, 16):
                        bi = tc_ - (16 - nb1)
                        nc.tensor.matmul(
                            ps[:, :N],
                            w116_sb[:, bi],
                            x16_sb[j][:, tc_ - 2 * mA],
                            start=(i == 0), stop=(i == nmm - 1))
                        i += 1
                    mB = mBs[j]
                    if c // 2 < mB:
                        dst = h8_sb[j][:, c // 2, c % 2]
                    else:
                        dst = h16_sb[j][:, c - 2 * mB]
                    nc.scalar.activation(dst, ps[:, :N], AF.Gelu,
                                         bias=b1_sb[:, c:c + 1],
                                         scale=1.0 / WSCALE)

            # phase B; weight slabs stream as half-pieces (SBUF pressure):
            # w28 pieces cover cc halves [0,16) / [16,32); w216 pieces cover
            # bf16-row halves, the LATER half first (boundary chunks with
            # mB > 0 read only high rows, and chunk order is ascending gate).
            hb2 = max(1, nb2 // 2)
            for d in range(16):
                w28p = []
                for piece, ring in ((0, nc.sync), (1, nc.scalar)):
                    w28_sb = w2p8.tile([128, 16, 2, 128], FP8, name="w28s",
                                       tag="w28s")
                    ring.dma_start(w28_sb[:], w2t8[d, :, 16 * piece:
                                                   16 * (piece + 1)])
                    w28p.append(w28_sb)
                w216p = [None, None]
                if w2t16 is not None:
                    pieces = (((1, nc.sync), (0, nc.scalar)) if nb2 > 1
                              else ((0, nc.sync),))
                    for piece, ring in pieces:
                        w216_sb = w2p16.tile([128, hb2, 128], BF,
                                             name="w216s", tag="w216s")
                        ring.dma_start(
                            w216_sb[:], w2t16[d, :, hb2 * piece:
                                              hb2 * (piece + 1)])
                        w216p[piece] = w216_sb
                for j, (off, N) in enumerate(chunks):
                    mB = mBs[j]
                    pso = psp.tile([128, 512], F32, name="pso", tag="ps")
                    nmm = mB + (64 - 2 * mB)
                    i = 0
                    for cc in range(mB):
                        nc.tensor.matmul(
                            pso[:, :N], w28p[cc // 16][:, cc % 16],
                            h8_sb[j][:, cc],
                            start=(i == 0), stop=(i == nmm - 1), perf_mode=DR)
                        i += 1
                    for ccc in range(2 * mB, 64):
                        bi = ccc - (64 - nb2)
                        nc.tensor.matmul(
                            pso[:, :N],
                            w216p[bi // hb2][:, bi % hb2],
                            h16_sb[j][:, ccc - 2 * mB],
                            start=(i == 0), stop=(i == nmm - 1))
                        i += 1
                    o2_sb = op.tile([128, 512], BF, name="o2", tag="o2")
                    nc.vector.scalar_tensor_tensor(
                        o2_sb[:, :N], pso[:, :N], b2_sb[:, d:d + 1],
                        wtok_sb[:, off: off + N],
                        op0=mybir.AluOpType.add, op1=mybir.AluOpType.mult)
                    oring = (nc.gpsimd if d < 15 else
                             (nc.sync, nc.scalar, nc.gpsimd)[j % 3])
                    oring.dma_start(out[d, :, off: off + N], o2_sb[:, :N])

    nc.compile()
    return nc


def _gelu_exact(x):
    x = np.asarray(x, np.float32)
    erf = np.vectorize(math.erf, otypes=[np.float32])
    return (x * np.float32(0.5) *
            (np.float32(1.0) + erf(x.astype(np.float64) / math.sqrt(2.0))))


def _sigmoid(x):
    x64 = np.asarray(x, np.float64)
    return (1.0 / (1.0 + np.exp(-x64))).astype(np.float32)


def _route(hidden, router_weight, router_bias, comp_w1, comp_b1, comp_w2,
           comp_b2):
    """Host replica of the reference routing: returns (k, indices, gates)."""
    pooled = hidden.mean(axis=1, dtype=np.float32)               # [B, D]
    ch = _gelu_exact(pooled @ comp_w1 + comp_b1)                 # [B, DC]
    complexity = _sigmoid(ch @ comp_w2 + comp_b2)                # [B, 1]
    capacity = float(np.mean(np.float32(MIN_CAP) +
                             complexity * np.float32(MAX_CAP - MIN_CAP)))
    k = int(capacity * S)
    if k == 0:
        return 0, None, None
    logits = (hidden.reshape(-1, D) @ router_weight).reshape(B, S)
    logits = logits + router_bias[0]                             # [B, S]
    if k >= S:
        idx = np.broadcast_to(np.arange(S, dtype=np.int64), (B, S)).copy()
    else:
        idx = np.argpartition(logits, S - k, axis=1)[:, S - k:]  # [B, k]
    gates = _sigmoid(np.take_along_axis(logits, idx, axis=1))    # [B, k]
    return k, idx, gates


def _q8(x):
    return np.clip(np.asarray(x, np.float32), -240.0, 240.0).astype(F8)


def _make_ntff_hook(so_path="/opt/axon/libaxon_pjrt.so"):
    """NTFF profiling hook driving the axon .so directly via ctypes."""
    import contextlib
    import ctypes
    import os
    if not os.path.exists(so_path):
        return None
    lib = ctypes.CDLL(so_path)
    if not hasattr(lib, "axon_start_nrt_profile"):
        return None
    lib.axon_start_nrt_profile.argtypes = [
        ctypes.POINTER(ctypes.c_int64), ctypes.c_size_t]
    lib.axon_start_nrt_profile.restype = ctypes.c_int64
    lib.axon_stop_nrt_profile.argtypes = [ctypes.c_char_p]
    lib.axon_stop_nrt_profile.restype = ctypes.c_int64

    @contextlib.contextmanager
    def _hook(output_dir, device_ids):
        import jax
        jax.devices()
        if device_ids:
            ids = (ctypes.c_int64 * len(device_ids))(*device_ids)
            rc = lib.axon_start_nrt_profile(ids, len(device_ids))
        else:
            rc = lib.axon_start_nrt_profile(None, 0)
        if rc != 0:
            raise RuntimeError(f"axon_start_nrt_profile rc={rc}")
        try:
            yield
        finally:
            n = lib.axon_stop_nrt_profile(str(output_dir).encode())
            print(f"profile: {n} file(s) written to {output_dir}",
                  file=sys.stderr)

    return _hook


def _ensure_axon_hooks():
    try:
        from antenv.axon_hooks import (get_axon_ntff_profile_hook,
                                       set_axon_ntff_profile_hook)
        if get_axon_ntff_profile_hook() is None:
            set_axon_ntff_profile_hook(_make_ntff_hook())
    except ImportError:
        import types
        mod = types.ModuleType("antenv.axon_hooks")
        mod._hook = _make_ntff_hook()
        mod.set_axon_ntff_profile_hook = lambda h: setattr(mod, "_hook", h)
        mod.get_axon_ntff_profile_hook = lambda: mod._hook
        try:
            import antenv
        except ImportError:
            antenv = types.ModuleType("antenv")
            sys.modules["antenv"] = antenv
        sys.modules["antenv.axon_hooks"] = mod
        antenv.axon_hooks = mod


def _run(inputs, trace=False):
    _ensure_axon_hooks()
    from concourse.bass_utils import run_bass_kernel_spmd

    hidden = np.ascontiguousarray(np.asarray(inputs["hidden"], np.float32))
    router_weight = np.asarray(inputs["router_weight"], np.float32)
    router_bias = np.asarray(inputs["router_bias"], np.float32)
    comp_w1 = np.asarray(inputs["comp_w1"], np.float32)
    comp_b1 = np.asarray(inputs["comp_b1"], np.float32)
    comp_w2 = np.asarray(inputs["comp_w2"], np.float32)
    comp_b2 = np.asarray(inputs["comp_b2"], np.float32)
    ffn_w1 = np.asarray(inputs["ffn_w1"], np.float32)
    ffn_b1 = np.asarray(inputs["ffn_b1"], np.float32)
    ffn_w2 = np.asarray(inputs["ffn_w2"], np.float32)
    ffn_b2 = np.asarray(inputs["ffn_b2"], np.float32)

    k, idx, gates = _route(hidden, router_weight, router_bias, comp_w1,
                           comp_b1, comp_w2, comp_b2)
    if k == 0:
        return hidden.copy(), None

    ntot = B * k
    n_tok = -(-ntot // NCORES)
    n_tok = -(-n_tok // TPAD) * TPAD
    npad = NCORES * n_tok

    selected = np.take_along_axis(hidden, idx[:, :, None], axis=1)  # [B,k,D]
    tokens = np.zeros((npad, D), np.float32)
    tokens[:ntot] = selected.reshape(ntot, D)
    gate_flat = np.zeros((npad,), np.float32)
    gate_flat[:ntot] = gates.reshape(ntot)

    # sort by gate ascending, deal round-robin to cores; each core's local
    # token order is then also gate-ascending with the same quantile profile
    order = np.argsort(gate_flat, kind="stable")
    perm = np.concatenate([order[c::NCORES] for c in range(NCORES)])
    tokens_s = tokens[perm]
    gate_s = gate_flat[perm]
    layout = _layout(n_tok, gate_flat[order])
    chunks = [(o, n) for (o, n, a, b) in layout]

    w1s = _q8(WSCALE * ffn_w1)
    w2s = _q8(WSCALE * ffn_w2)
    w1t8 = w1s.reshape(8, 2, 128, 64, 128).transpose(3, 2, 0, 1, 4)
    w2t8 = w2s.reshape(32, 2, 128, 16, 128).transpose(3, 2, 0, 1, 4)
    if SWI:
        # software-interleaved LDWEIGHTS layout: per 256-col weight block,
        # mem[p, 2*(127-j)+o] = slot-o column j
        w1t8 = w1t8.transpose(0, 1, 2, 4, 3)[:, :, :, ::-1, :]
        w2t8 = w2t8.transpose(0, 1, 2, 4, 3)[:, :, :, ::-1, :]
        w1t8 = w1t8.reshape(64, 128, 8, 2, 128)
        w2t8 = w2t8.reshape(16, 128, 32, 2, 128)
    w1t8 = np.ascontiguousarray(w1t8)
    w2t8 = np.ascontiguousarray(w2t8)
    mAs = [a for (o, n, a, b) in layout]
    mBs = [b for (o, n, a, b) in layout]
    nb1 = 16 - 2 * min(mAs)
    nb2 = 64 - 2 * min(mBs)
    # bf16 tails share the PSUM accumulators with the x64-scaled fp8 parts,
    # so they carry the same pre-scale (undone by the 1/64 gelu scale / gate)
    w1b = (np.float32(WSCALE) * ffn_w1).astype(BF16)
    w2b = (np.float32(WSCALE) * ffn_w2).astype(BF16)
    # w1t16[c, p, t, j] = w1[(16-nb1+t)*128+p, c*128+j]
    w1t16 = (np.ascontiguousarray(
        w1b[(16 - nb1) * 128:].reshape(nb1, 128, 64, 128)
        .transpose(2, 1, 0, 3)) if nb1 else None)
    w2t16 = (np.ascontiguousarray(
        w2b[(64 - nb2) * 128:].reshape(nb2, 128, 16, 128)
        .transpose(2, 1, 0, 3)) if nb2 else None)
    b1t = np.ascontiguousarray(ffn_b1.reshape(64, 128).T)
    b2t = np.ascontiguousarray(np.float32(WSCALE) * ffn_b2.reshape(16, 128).T)

    in_maps = []
    for c in range(NCORES):
        tok_c = tokens_s[c * n_tok:(c + 1) * n_tok]              # [n, D]
        im = {"w1t8": w1t8, "w2t8": w2t8, "b1t": b1t, "b2t": b2t}
        if w1t16 is not None:
            im["w1t16"] = w1t16
        if w2t16 is not None:
            im["w2t16"] = w2t16
        for j, (off, N) in enumerate(chunks):
            mA = mAs[j]
            blk = tok_c[off:off + N]                             # [N, D]
            if mA:
                im[f"x8_{j}"] = np.ascontiguousarray(
                    _q8(blk[:, :mA * 256]).reshape(N, mA, 2, 128)
                    .transpose(3, 1, 2, 0))
            if mA < 8:
                im[f"x16_{j}"] = np.ascontiguousarray(
                    blk[:, mA * 256:].astype(BF16).reshape(N, 16 - 2 * mA, 128)
                    .transpose(2, 1, 0))
        im["wtok"] = np.ascontiguousarray(np.broadcast_to(
            (gate_s[c * n_tok:(c + 1) * n_tok] /
             np.float32(WSCALE)).astype(BF16)[None], (128, n_tok)))
        in_maps.append(im)

    key = (n_tok, layout)
    if key not in _cache:
        _cache[key] = _build(n_tok, layout)
    nc = _cache[key]

    last_err = None
    for attempt in range(3):
        try:
            res = run_bass_kernel_spmd(nc, in_maps,
                                       core_ids=list(range(NCORES)),
                                       trace=trace)
            break
        except Exception as e:  # noqa: BLE001
            last_err = e
            import time
            time.sleep(3.0 * (attempt + 1))
    else:
        raise last_err

    weighted_s = np.empty((npad, D), np.float32)
    for c in range(NCORES):
        o = res.results[c]["out"]                                # [16,128,n]
        weighted_s[c * n_tok:(c + 1) * n_tok] = \
            o.astype(np.float32).reshape(D, n_tok).T
    weighted = np.empty((npad, D), np.float32)
    weighted[perm] = weighted_s
    weighted = weighted[:ntot].reshape(B, k, D)

    output = hidden.copy()
    b_idx = np.arange(B)[:, None]
    output[b_idx, idx] += weighted
    return output, res.exec_time_ns


def kernel(**inputs):
    output, _ = _run(inputs, trace=False)
    return output



# revision 5
# speedup vs baseline: 1.0204x; 1.0204x over previous
"""AdaptiveMoDBlock Trainium2 kernel — gate-aware mixed fp8/fp16 variant.

Host: routing/top-k as in the reference; selected tokens are then SORTED by
gate ascending and dealt round-robin to the 8 cores (each core sees the same
gate-quantile profile, so one SPMD graph fits all).  The output error a token
contributes scales with gate^2, so low-gate tokens run the inner FFN in e4m3
DoubleRow (2 MACs/PE/cycle) and high-gate tokens in fp16; a boundary chunk
mixes precision across the contraction (first mA/mB k-doubles fp8, rest fp16)
to hit the error budget exactly.

Precision allocation: per 256-token block (gate-ascending), choose
(mA of 8, mB of 32) = number of 256-row k-doubles computed in fp8 for the
two FFN matmuls by a calibrated greedy: each fp8 k-double unit adds
err^2 ~= ALPHA_{A,B} * (block gate^2 mass) and saves 64 (phase A) or 16
(phase B) matmul slots; fill cheapest err^2-per-ns first against the
TARGET_REL budget.

Device layouts per chunk j (width Nj <= 512):
  x8_j  f8   [128, mA_j, 2, Nj]    x8[p,t,o,n]  = sel[n, (2t+o)*128+p]
  x16_j f16  [128, 16-2mA_j, Nj]   rows 2mA_j..15
  h8_j  f8   [128, mB_j, 2, Nj]    dff rows < 2mB_j*128 (phase-A output)
  h16_j f16  [128, 64-2mB_j, Nj]   dff rows >= 2mB_j*128
Weights (shared): w1t8/w2t8 fp8 (x64 pre-scale), w1t16/w2t16 fp16 tails.
"""

import math
import sys

import numpy as np

sys.path.insert(0, "/opt/trn_rl_repo")

import ml_dtypes  # noqa: E402

F16 = np.float16
F8 = ml_dtypes.float8_e4m3   # IEEE e4m3 (max 240) == TRN FP8_EXP4

B, S, D = 4, 4096, 2048
DC = D // 4
DFF = 4 * D
MIN_CAP, MAX_CAP = 0.25, 1.0

NCORES = 8
TPAD = 256
WSCALE = 64.0
SWI = True           # software-interleaved DoubleRow weight layout

# calibrated error model (err^2 units of ||out - expected||_F^2):
#   one mA unit (256 fp8 contraction rows, phase A) on a block with gate^2
#   mass m adds ALPHA_A*m; one mB unit (phase B) adds ALPHA_B*m; the
#   fp16-path floor adds EPS_FIX*m per block.  ||expected||^2 ~=
#   ||hidden||^2 + NORM2_C * sum(gate^2).
ALPHA_A = 0.4262
ALPHA_B = 0.0996
EPS_FIX = 0.0006
NORM2_C = 2246.0
TARGET_REL = 0.0198
T_UNIT_A = 4.0       # phase-A unit time saved / phase-B unit time saved

_cache = {}


def _choose_cfg(n_tok, gate_sorted, hidden_norm2):
    """Per-256-block (mA, mB) by calibrated greedy against TARGET_REL."""
    nblk = n_tok // 256
    g2 = gate_sorted.astype(np.float64) ** 2
    tot_mass = g2.sum()
    if tot_mass <= 0:
        return tuple((8, 32) for _ in range(nblk))
    bs = NCORES * 256
    m = [float(g2[j * bs:(j + 1) * bs].sum()) for j in range(nblk)]
    norm2 = float(hidden_norm2) + NORM2_C * float(tot_mass)
    budget = TARGET_REL * TARGET_REL * norm2 - EPS_FIX * float(tot_mass)
    # unit costs: (err2, time-weight, block, phase, max-units)
    units = []
    for j in range(nblk):
        units.append([ALPHA_B * m[j], 1.0, j, "B", 32])
        units.append([ALPHA_A * m[j], T_UNIT_A, j, "A", 8])
    units.sort(key=lambda u: u[0] / u[1])
    mA = [0] * nblk
    mB = [0] * nblk
    spent = 0.0
    for cost, tw, j, ph, mx in units:
        n = min(mx, int((budget - spent) / cost)) if cost > 0 else mx
        if n <= 0:
            continue
        spent += n * cost
        if ph == "A":
            mA[j] = n
        else:
            mB[j] = n
    return tuple(zip(mA, mB))


def _layout(n_tok, gate_sorted, hidden_norm2):
    """Merge adjacent same-precision 256-blocks into <=512-wide chunks."""
    cfg = _choose_cfg(n_tok, gate_sorted, hidden_norm2)
    merged = []
    for j, (mA, mB) in enumerate(cfg):
        off = j * 256
        if (merged and merged[-1][2] == mA and merged[-1][3] == mB
                and merged[-1][1] + 256 <= 512
                and merged[-1][0] + merged[-1][1] == off):
            merged[-1] = (merged[-1][0], merged[-1][1] + 256, mA, mB)
        else:
            merged.append((off, 256, mA, mB))
    return tuple(merged)


def _build(n_tok, layout):
    """Build + compile the per-core FFN graph."""
    from concourse import bacc, mybir, tile

    F32 = mybir.dt.float32
    FP8 = mybir.dt.float8e4
    HF = mybir.dt.float16
    AF = mybir.ActivationFunctionType
    DR = (mybir.MatmulPerfMode.DoubleRowSwInterleave if SWI else
          mybir.MatmulPerfMode.DoubleRow)

    chunks = [(o, n) for (o, n, a, b) in layout]
    nch = len(chunks)
    mAs = [a for (o, n, a, b) in layout]
    mBs = [b for (o, n, a, b) in layout]
    nb1 = 16 - 2 * min(mAs)          # fp16 w1 rows (in 128-chunks)
    nb2 = 64 - 2 * min(mBs)          # fp16 w2 rows

    nc = bacc.Bacc("TRN2", target_bir_lowering=False, debug=False,
                   num_devices=NCORES)
    x8t, x16t = [], []
    for j, (off, N) in enumerate(chunks):
        mA = mAs[j]
        x8t.append(nc.declare_dram_parameter(f"x8_{j}", [128, mA, 2, N], FP8,
                                             isOutput=False) if mA else None)
        x16t.append(nc.declare_dram_parameter(
            f"x16_{j}", [128, 16 - 2 * mA, N], HF,
            isOutput=False) if mA < 8 else None)
    w1t8 = nc.declare_dram_parameter("w1t8", [64, 128, 8, 2, 128], FP8,
                                     isOutput=False)
    w1t16 = (nc.declare_dram_parameter("w1t16", [64, 128, nb1, 128], HF,
                                       isOutput=False) if nb1 else None)
    w2t8 = nc.declare_dram_parameter("w2t8", [16, 128, 32, 2, 128], FP8,
                                     isOutput=False)
    w2t16 = (nc.declare_dram_parameter("w2t16", [16, 128, nb2, 128], HF,
                                       isOutput=False) if nb2 else None)
    b1t = nc.declare_dram_parameter("b1t", [128, 64], F32, isOutput=False)
    b2t = nc.declare_dram_parameter("b2t", [128, 16], F32, isOutput=False)
    wtok = nc.declare_dram_parameter("wtok", [128, n_tok], HF, isOutput=False)
    out = nc.declare_dram_parameter("out", [16, 128, n_tok], HF,
                                    isOutput=True)

    with tile.TileContext(nc) as tc:
        with (
            tc.tile_pool(name="const", bufs=1) as cpool,
            tc.tile_pool(name="w1p8", bufs=3) as w1p8,
            tc.tile_pool(name="w1p16", bufs=3) as w1p16,
            tc.tile_pool(name="w2p8", bufs=3) as w2p8,
            tc.tile_pool(name="w2p16", bufs=3) as w2p16,
            tc.tile_pool(name="op", bufs=2) as op,
            tc.tile_pool(name="ps", bufs=8, space="PSUM") as psp,
        ):
            x8_sb, x16_sb, h8_sb, h16_sb = [], [], [], []
            for j, (off, N) in enumerate(chunks):
                mA, mB = mAs[j], mBs[j]
                x8_sb.append(cpool.tile([128, mA, 2, N], FP8, name=f"x8s{j}",
                                        tag=f"x8s{j}") if mA else None)
                x16_sb.append(cpool.tile([128, 16 - 2 * mA, N], HF,
                                         name=f"x16s{j}", tag=f"x16s{j}")
                              if mA < 8 else None)
                h8_sb.append(cpool.tile([128, mB, 2, N], FP8, name=f"h8s{j}",
                                        tag=f"h8s{j}") if mB else None)
                h16_sb.append(cpool.tile([128, 64 - 2 * mB, N], HF,
                                         name=f"h16s{j}", tag=f"h16s{j}")
                              if mB < 32 else None)

            # ramp DMA: w1 slab 0 first on sync, then x pieces round-robin
            # over the sync/scalar/vector HWDGE rings in PE consumption
            # order (chunk-major, k ascending); w116 preloads + small
            # constants ride the gpsimd SWDGE ring so they never queue
            # ahead of the c=0 chain.
            w18_pre, w116_pre = {}, {}
            t8 = w1p8.tile([128, 8, 2, 128], FP8, name="w18s", tag="w18s")
            nc.sync.dma_start(t8[:], w1t8[0])
            w18_pre[0] = t8
            pieces = []
            for j in range(nch):
                mA = mAs[j]
                for t in range(mA):
                    pieces.append((x8_sb[j][:, t], x8t[j][:, t]))
                if mA < 8:
                    for t in range(16 - 2 * mA):
                        pieces.append((x16_sb[j][:, t], x16t[j][:, t]))
            xrings = [nc.scalar, nc.sync]
            for ri, (dst, srcp) in enumerate(pieces):
                xrings[ri % 2].dma_start(dst, srcp)
            t8 = w1p8.tile([128, 8, 2, 128], FP8, name="w18s", tag="w18s")
            nc.gpsimd.dma_start(t8[:], w1t8[1])
            w18_pre[1] = t8
            if w1t16 is not None:
                for c in (0, 1):
                    t16 = w1p16.tile([128, nb1, 128], HF, name="w116s",
                                     tag="w116s")
                    nc.gpsimd.dma_start(t16[:], w1t16[c])
                    w116_pre[c] = t16
            b1_sb = cpool.tile([128, 64], F32, tag="b1")
            nc.gpsimd.dma_start(b1_sb[:], b1t[:])
            wtok_sb = cpool.tile([128, n_tok], HF, tag="wtok")
            nc.gpsimd.dma_start(wtok_sb[:], wtok[:])
            b2_sb = cpool.tile([128, 16], F32, tag="b2")
            nc.gpsimd.dma_start(b2_sb[:], b2t[:])

            # phase A; for the first two c's process the leading chunks
            # before the trailing ones so the trailing chunks' x DMA gets
            # ~4 extra matmul-bursts of slack during the ramp.
            head = list(range(min(2, nch)))
            tail = [j for j in range(nch) if j not in head]
            sched = ([(c, j) for c in range(min(2, 64)) for j in head]
                     + [(c, j) for c in range(min(2, 64)) for j in tail]
                     + [(c, j) for c in range(2, 64) for j in range(nch)])
            w18_cur, w116_cur = {}, {}
            for c, j in sched:
                if c not in w18_cur:
                    if c in w18_pre:
                        w18_cur[c] = w18_pre.pop(c)
                        w116_cur[c] = w116_pre.pop(c, None)
                    else:
                        w18_sb = w1p8.tile([128, 8, 2, 128], FP8,
                                           name="w18s", tag="w18s")
                        nc.sync.dma_start(w18_sb[:], w1t8[c])
                        w18_cur[c] = w18_sb
                        w116_cur[c] = None
                        if w1t16 is not None:
                            w116_sb = w1p16.tile([128, nb1, 128], HF,
                                                 name="w116s", tag="w116s")
                            ring = nc.scalar if c % 2 == 0 else nc.gpsimd
                            ring.dma_start(w116_sb[:], w1t16[c])
                            w116_cur[c] = w116_sb
                w18_sb = w18_cur[c]
                w116_sb = w116_cur[c]
                if True:
                    off, N = chunks[j]
                    mA = mAs[j]
                    ps = psp.tile([128, 512], F32, name="ps", tag="ps")
                    nmm = mA + (16 - 2 * mA)
                    i = 0
                    for t in range(mA):
                        nc.tensor.matmul(
                            ps[:, :N], w18_sb[:, t], x8_sb[j][:, t],
                            start=(i == 0), stop=(i == nmm - 1), perf_mode=DR)
                        i += 1
                    for tc_ in range(2 * mA, 16):
                        bi = tc_ - (16 - nb1)
                        nc.tensor.matmul(
                            ps[:, :N],
                            w116_sb[:, bi],
                            x16_sb[j][:, tc_ - 2 * mA],
                            start=(i == 0), stop=(i == nmm - 1))
                        i += 1
                    mB = mBs[j]
                    if c // 2 < mB:
                        dst = h8_sb[j][:, c // 2, c % 2]
                    else:
                        dst = h16_sb[j][:, c - 2 * mB]
                    nc.scalar.activation(dst, ps[:, :N], AF.Gelu,
                                         bias=b1_sb[:, c:c + 1],
                                         scale=1.0 / WSCALE)

            # phase B; weight slabs stream as half-pieces (SBUF pressure):
            # w28 pieces cover cc halves [0,16) / [16,32); w216 pieces cover
            # fp16-row halves, the LATER half first (boundary chunks with
            # mB > 0 read only high rows, and chunk order is ascending gate).
            hb2 = max(1, nb2 // 2)
            for d in range(16):
                w28p = []
                for piece, ring in ((0, nc.sync), (1, nc.scalar)):
                    w28_sb = w2p8.tile([128, 16, 2, 128], FP8, name="w28s",
                                       tag="w28s")
                    ring.dma_start(w28_sb[:], w2t8[d, :, 16 * piece:
                                                   16 * (piece + 1)])
                    w28p.append(w28_sb)
                w216p = [None, None]
                if w2t16 is not None:
                    pieces = (((1, nc.sync), (0, nc.scalar)) if nb2 > 1
                              else ((0, nc.sync),))
                    for piece, ring in pieces:
                        w216_sb = w2p16.tile([128, hb2, 128], HF,
                                             name="w216s", tag="w216s")
                        ring.dma_start(
                            w216_sb[:], w2t16[d, :, hb2 * piece:
                                              hb2 * (piece + 1)])
                        w216p[piece] = w216_sb
                for j, (off, N) in enumerate(chunks):
                    mB = mBs[j]
                    pso = psp.tile([128, 512], F32, name="pso", tag="ps")
                    nmm = mB + (64 - 2 * mB)
                    i = 0
                    for cc in range(mB):
                        nc.tensor.matmul(
                            pso[:, :N], w28p[cc // 16][:, cc % 16],
                            h8_sb[j][:, cc],
                            start=(i == 0), stop=(i == nmm - 1), perf_mode=DR)
                        i += 1
                    for ccc in range(2 * mB, 64):
                        bi = ccc - (64 - nb2)
                        nc.tensor.matmul(
                            pso[:, :N],
                            w216p[bi // hb2][:, bi % hb2],
                            h16_sb[j][:, ccc - 2 * mB],
                            start=(i == 0), stop=(i == nmm - 1))
                        i += 1
                    o2_sb = op.tile([128, 512], HF, name="o2", tag="o2")
                    nc.vector.scalar_tensor_tensor(
                        o2_sb[:, :N], pso[:, :N], b2_sb[:, d:d + 1],
                        wtok_sb[:, off: off + N],
                        op0=mybir.AluOpType.add, op1=mybir.AluOpType.mult)
                    oring = (nc.gpsimd if d < 15 else
                             (nc.sync, nc.scalar, nc.gpsimd)[j % 3])
                    oring.dma_start(out[d, :, off: off + N], o2_sb[:, :N])

    nc.compile()
    return nc


def _gelu_exact(x):
    x = np.asarray(x, np.float32)
    erf = np.vectorize(math.erf, otypes=[np.float32])
    return (x * np.float32(0.5) *
            (np.float32(1.0) + erf(x.astype(np.float64) / math.sqrt(2.0))))


def _sigmoid(x):
    x64 = np.asarray(x, np.float64)
    return (1.0 / (1.0 + np.exp(-x64))).astype(np.float32)


def _route(hidden, router_weight, router_bias, comp_w1, comp_b1, comp_w2,
           comp_b2):
    """Host replica of the reference routing: returns (k, indices, gates)."""
    pooled = hidden.mean(axis=1, dtype=np.float32)               # [B, D]
    ch = _gelu_exact(pooled @ comp_w1 + comp_b1)                 # [B, DC]
    complexity = _sigmoid(ch @ comp_w2 + comp_b2)                # [B, 1]
    capacity = float(np.mean(np.float32(MIN_CAP) +
                             complexity * np.float32(MAX_CAP - MIN_CAP)))
    k = int(capacity * S)
    if k == 0:
        return 0, None, None
    logits = (hidden.reshape(-1, D) @ router_weight).reshape(B, S)
    logits = logits + router_bias[0]                             # [B, S]
    if k >= S:
        idx = np.broadcast_to(np.arange(S, dtype=np.int64), (B, S)).copy()
    else:
        idx = np.argpartition(logits, S - k, axis=1)[:, S - k:]  # [B, k]
    gates = _sigmoid(np.take_along_axis(logits, idx, axis=1))    # [B, k]
    return k, idx, gates


def _q8(x):
    return np.clip(np.asarray(x, np.float32), -240.0, 240.0).astype(F8)


def _make_ntff_hook(so_path="/opt/axon/libaxon_pjrt.so"):
    """NTFF profiling hook driving the axon .so directly via ctypes."""
    import contextlib
    import ctypes
    import os
    if not os.path.exists(so_path):
        return None
    lib = ctypes.CDLL(so_path)
    if not hasattr(lib, "axon_start_nrt_profile"):
        return None
    lib.axon_start_nrt_profile.argtypes = [
        ctypes.POINTER(ctypes.c_int64), ctypes.c_size_t]
    lib.axon_start_nrt_profile.restype = ctypes.c_int64
    lib.axon_stop_nrt_profile.argtypes = [ctypes.c_char_p]
    lib.axon_stop_nrt_profile.restype = ctypes.c_int64

    @contextlib.contextmanager
    def _hook(output_dir, device_ids):
        import jax
        jax.devices()
        if device_ids:
            ids = (ctypes.c_int64 * len(device_ids))(*device_ids)
            rc = lib.axon_start_nrt_profile(ids, len(device_ids))
        else:
            rc = lib.axon_start_nrt_profile(None, 0)
        if rc != 0:
            raise RuntimeError(f"axon_start_nrt_profile rc={rc}")
        try:
            yield
        finally:
            n = lib.axon_stop_nrt_profile(str(output_dir).encode())
            print(f"profile: {n} file(s) written to {output_dir}",
                  file=sys.stderr)

    return _hook


def _ensure_axon_hooks():
    try:
        from antenv.axon_hooks import (get_axon_ntff_profile_hook,
                                       set_axon_ntff_profile_hook)
        if get_axon_ntff_profile_hook() is None:
            set_axon_ntff_profile_hook(_make_ntff_hook())
    except ImportError:
        import types
        mod = types.ModuleType("antenv.axon_hooks")
        mod._hook = _make_ntff_hook()
        mod.set_axon_ntff_profile_hook = lambda h: setattr(mod, "_hook", h)
        mod.get_axon_ntff_profile_hook = lambda: mod._hook
        try:
            import antenv
        except ImportError:
            antenv = types.ModuleType("antenv")
            sys.modules["antenv"] = antenv
        sys.modules["antenv.axon_hooks"] = mod
        antenv.axon_hooks = mod


def _run(inputs, trace=False):
    _ensure_axon_hooks()
    from concourse.bass_utils import run_bass_kernel_spmd

    hidden = np.ascontiguousarray(np.asarray(inputs["hidden"], np.float32))
    router_weight = np.asarray(inputs["router_weight"], np.float32)
    router_bias = np.asarray(inputs["router_bias"], np.float32)
    comp_w1 = np.asarray(inputs["comp_w1"], np.float32)
    comp_b1 = np.asarray(inputs["comp_b1"], np.float32)
    comp_w2 = np.asarray(inputs["comp_w2"], np.float32)
    comp_b2 = np.asarray(inputs["comp_b2"], np.float32)
    ffn_w1 = np.asarray(inputs["ffn_w1"], np.float32)
    ffn_b1 = np.asarray(inputs["ffn_b1"], np.float32)
    ffn_w2 = np.asarray(inputs["ffn_w2"], np.float32)
    ffn_b2 = np.asarray(inputs["ffn_b2"], np.float32)

    k, idx, gates = _route(hidden, router_weight, router_bias, comp_w1,
                           comp_b1, comp_w2, comp_b2)
    if k == 0:
        return hidden.copy(), None

    ntot = B * k
    n_tok = -(-ntot // NCORES)
    n_tok = -(-n_tok // TPAD) * TPAD
    npad = NCORES * n_tok

    selected = np.take_along_axis(hidden, idx[:, :, None], axis=1)  # [B,k,D]
    tokens = np.zeros((npad, D), np.float32)
    tokens[:ntot] = selected.reshape(ntot, D)
    gate_flat = np.zeros((npad,), np.float32)
    gate_flat[:ntot] = gates.reshape(ntot)

    # sort by gate ascending, deal round-robin to cores; each core's local
    # token order is then also gate-ascending with the same quantile profile
    order = np.argsort(gate_flat, kind="stable")
    perm = np.concatenate([order[c::NCORES] for c in range(NCORES)])
    tokens_s = tokens[perm]
    gate_s = gate_flat[perm]
    hidden_norm2 = float((hidden.astype(np.float64) ** 2).sum())
    layout = _layout(n_tok, gate_flat[order], hidden_norm2)
    chunks = [(o, n) for (o, n, a, b) in layout]

    w1s = _q8(WSCALE * ffn_w1)
    w2s = _q8(WSCALE * ffn_w2)
    w1t8 = w1s.reshape(8, 2, 128, 64, 128).transpose(3, 2, 0, 1, 4)
    w2t8 = w2s.reshape(32, 2, 128, 16, 128).transpose(3, 2, 0, 1, 4)
    if SWI:
        # software-interleaved LDWEIGHTS layout: per 256-col weight block,
        # mem[p, 2*(127-j)+o] = slot-o column j
        w1t8 = w1t8.transpose(0, 1, 2, 4, 3)[:, :, :, ::-1, :]
        w2t8 = w2t8.transpose(0, 1, 2, 4, 3)[:, :, :, ::-1, :]
        w1t8 = w1t8.reshape(64, 128, 8, 2, 128)
        w2t8 = w2t8.reshape(16, 128, 32, 2, 128)
    w1t8 = np.ascontiguousarray(w1t8)
    w2t8 = np.ascontiguousarray(w2t8)
    mAs = [a for (o, n, a, b) in layout]
    mBs = [b for (o, n, a, b) in layout]
    nb1 = 16 - 2 * min(mAs)
    nb2 = 64 - 2 * min(mBs)
    # fp16 tails share the PSUM accumulators with the x64-scaled fp8 parts,
    # so they carry the same pre-scale (undone by the 1/64 gelu scale / gate)
    w1b = (np.float32(WSCALE) * ffn_w1).astype(F16)
    w2b = (np.float32(WSCALE) * ffn_w2).astype(F16)
    # w1t16[c, p, t, j] = w1[(16-nb1+t)*128+p, c*128+j]
    w1t16 = (np.ascontiguousarray(
        w1b[(16 - nb1) * 128:].reshape(nb1, 128, 64, 128)
        .transpose(2, 1, 0, 3)) if nb1 else None)
    w2t16 = (np.ascontiguousarray(
        w2b[(64 - nb2) * 128:].reshape(nb2, 128, 16, 128)
        .transpose(2, 1, 0, 3)) if nb2 else None)
    b1t = np.ascontiguousarray(ffn_b1.reshape(64, 128).T)
    b2t = np.ascontiguousarray(np.float32(WSCALE) * ffn_b2.reshape(16, 128).T)

    in_maps = []
    for c in range(NCORES):
        tok_c = tokens_s[c * n_tok:(c + 1) * n_tok]              # [n, D]
        im = {"w1t8": w1t8, "w2t8": w2t8, "b1t": b1t, "b2t": b2t}
        if w1t16 is not None:
            im["w1t16"] = w1t16
        if w2t16 is not None:
            im["w2t16"] = w2t16
        for j, (off, N) in enumerate(chunks):
            mA = mAs[j]
            blk = tok_c[off:off + N]                             # [N, D]
            if mA:
                im[f"x8_{j}"] = np.ascontiguousarray(
                    _q8(blk[:, :mA * 256]).reshape(N, mA, 2, 128)
                    .transpose(3, 1, 2, 0))
            if mA < 8:
                im[f"x16_{j}"] = np.ascontiguousarray(
                    blk[:, mA * 256:].astype(F16).reshape(N, 16 - 2 * mA, 128)
                    .transpose(2, 1, 0))
        im["wtok"] = np.ascontiguousarray(np.broadcast_to(
            (gate_s[c * n_tok:(c + 1) * n_tok] /
             np.float32(WSCALE)).astype(F16)[None], (128, n_tok)))
        in_maps.append(im)

    key = (n_tok, layout)
    if key not in _cache:
        _cache[key] = _build(n_tok, layout)
    nc = _cache[key]

    last_err = None
    for attempt in range(3):
        try:
            res = run_bass_kernel_spmd(nc, in_maps,
                                       core_ids=list(range(NCORES)),
                                       trace=trace)
            break
        except Exception as e:  # noqa: BLE001
            last_err = e
            import time
            time.sleep(3.0 * (attempt + 1))
    else:
        raise last_err

    weighted_s = np.empty((npad, D), np.float32)
    for c in range(NCORES):
        o = res.results[c]["out"]                                # [16,128,n]
        weighted_s[c * n_tok:(c + 1) * n_tok] = \
            o.astype(np.float32).reshape(D, n_tok).T
    weighted = np.empty((npad, D), np.float32)
    weighted[perm] = weighted_s
    weighted = weighted[:ntot].reshape(B, k, D)

    output = hidden.copy()
    b_idx = np.arange(B)[:, None]
    output[b_idx, idx] += weighted
    return output, res.exec_time_ns


def kernel(**inputs):
    output, _ = _run(inputs, trace=False)
    return output
